# revision 79
# baseline (speedup 1.0000x reference)
"""Trainium2 Bass kernel for MimiAttention (GQA + RoPE + causal softmax).

Problem: B=2, S=2048, H=1024, NH=16 q-heads, NKV=4 kv-heads, HD=64.
Sharding: 8 cores = 2 (batch) x 4 (kv-group).  Each core computes one batch's
attention for one GQA group (4 q-heads sharing 1 kv head) and the partial
o-projection for those heads; the host sums the 4 partials per batch.

Per-core device pipeline (all matmuls bf16 in / fp32 psum out):
  1. QKV projection in [d, s] layout.  RoPE is realized without any
     cross-partition shuffles by computing a second projection with
     sign-permuted weight rows (W2 rows: d<32 -> -W[d+32], d>=32 -> W[d-32]):
       q_rot = q*cos + q2*sin
     The scores contraction then uses the 128-dim identity
       q_rot . k_rot = concat(q*cos, q2*sin) . concat(k_rot, k_rot)
     so Qhat = [q*cos; q2*sin] needs only ONE elementwise multiply per chunk,
     and Khat = [k_rot; k_rot] is built by one matmul with the fold matrix
     J[p,m] = (p == m mod 64).
  2. Scores computed TRANSPOSED (scoresT[j,i]) per key-tile jt into wide
     [128,1024] psum tiles; ONE exp per tile on ACT with the 1/sqrt(64)
     scale folded in (no max subtraction needed: |s*scale|<~3); causal
     triangle handled by a bf16 mask multiply on DVE for the diagonal tile.
  3. attnV flipped: for each query tile `it`, accumulate over ALL key tiles
     jt<=it into a small [128,65] psum ring (column 64 = ones -> softmax
     denominator), then reciprocal+scale on DVE.  Short psum residency
     frees banks for double-buffered scores and o-proj.
  4. Software pipelining: attnV for tile it is issued two stages behind the
     scores for tile it+2, so PE never stalls on ACT's exp latency.
  5. o-projection per 512-col chunk: PE-transpose attn [i,c] -> [c,i]
     (4 transposes batched per psum tile), 2-slot psum ring, output copies
     alternate DVE/ACT, one batched DMA per chunk ([128,8,512] -> oT).
"""

import numpy as np
import ml_dtypes

B, S, H = 2, 2048, 1024
NH, NKV, HD = 16, 4, 64
G = NH // NKV            # 4 q-heads per kv head
THETA = 10000.0
N_CORES = 8

BF16 = ml_dtypes.bfloat16


def _build_nc(debug=False):
    import concourse.mybir as mybir
    import concourse.tile as tile
    from concourse import bacc

    f32 = mybir.dt.float32
    bf16 = mybir.dt.bfloat16

    nc = bacc.Bacc("TRN2", target_bir_lowering=False)

    xTd = nc.dram_tensor("xT", [H, S], bf16, kind="ExternalInput")
    wqkd = nc.dram_tensor("wqkT", [H, 256], bf16, kind="ExternalInput")
    wkd = nc.dram_tensor("wkT2", [128, 1024], bf16, kind="ExternalInput")
    wvd = nc.dram_tensor("wvT", [H, HD], bf16, kind="ExternalInput")
    csd = nc.dram_tensor("cs", [128, S], bf16, kind="ExternalInput")
    cs2d = nc.dram_tensor("cs2", [128, S], bf16, kind="ExternalInput")
    wod = nc.dram_tensor("woT", [G * HD, H], bf16, kind="ExternalInput")
    djd = nc.dram_tensor("dupJ", [128, 128], bf16, kind="ExternalInput")
    pqd = nc.dram_tensor("permQ", [128, 128], bf16, kind="ExternalInput")
    idd = nc.dram_tensor("ident", [128, 128], bf16, kind="ExternalInput")
    trid = nc.dram_tensor("trimask", [128, 128], bf16, kind="ExternalInput")
    oTd = nc.dram_tensor("oT", [H, S], bf16, kind="ExternalOutput")
    if debug:
        dbg_attn = nc.dram_tensor("dbg_attn", [2, 128, S], bf16,
                                  kind="ExternalOutput")
        dbg_exp = nc.dram_tensor("dbg_exp", [16, 128, S], bf16,
                                 kind="ExternalOutput")
        dbg_qk = nc.dram_tensor("dbg_qk", [G + 1, 128, S], bf16,
                                kind="ExternalOutput")
        dbg_v = nc.dram_tensor("dbg_v", [128, 16, HD + 1], bf16,
                               kind="ExternalOutput")
        dbg_aT = nc.dram_tensor("dbg_aT", [2, 128, S], bf16,
                                kind="ExternalOutput")

    NSB = S // 512        # 4 chunks of 512
    NST = S // 128        # 16 tiles of 128
    KC = H // 128         # 8 contraction chunks
    scale = float(1.0 / np.sqrt(HD))
    Exp = mybir.ActivationFunctionType.Exp

    with tile.TileContext(nc) as tc:
        import contextlib
        ctx = contextlib.ExitStack()
        with ctx:
            consts = ctx.enter_context(tc.tile_pool(name="consts", bufs=1))
            acts = ctx.enter_context(tc.tile_pool(name="acts", bufs=1))
            anp = ctx.enter_context(tc.tile_pool(name="attn", bufs=1))
            rcp = ctx.enter_context(tc.tile_pool(name="rcp", bufs=6))
            etp = ctx.enter_context(tc.tile_pool(name="etri", bufs=8))
            ep = ctx.enter_context(tc.tile_pool(name="exps", bufs=1))
            otp = ctx.enter_context(tc.tile_pool(name="ot", bufs=2))
            # PSUM: scores 2x[128,1024]f32 (4 banks) + attnV accum
            # 2x[128,65] (2 banks) + shared proj/oproj ring 2x[128,512]
            # (2 banks) = 8 banks.
            scp = ctx.enter_context(
                tc.tile_pool(name="sc", bufs=2, space="PSUM"))
            pav = ctx.enter_context(
                tc.tile_pool(name="av", bufs=2, space="PSUM"))
            pvp = ctx.enter_context(
                tc.tile_pool(name="vr", bufs=2, space="PSUM"))

            # ---- input DMAs, ordered by first use; xt arrives in 512-col
            # chunks (all KC row-blocks per chunk) so the prewave can start
            # after ~2 transfers.
            xtr = xTd.rearrange("(kc p) m -> p kc m", p=128)
            wqkr = wqkd.rearrange("(kc p) m -> p kc m", p=128)
            wk_sb = consts.tile([128, KC, 128], bf16, tag="wk")
            nc.sync.dma_start(wk_sb, wkd.rearrange("p (kc m) -> p kc m",
                                                   kc=KC))
            xt_sb = consts.tile([128, KC, S], bf16, tag="xt")
            nc.sync.dma_start(xt_sb[:, :, 0:512], xtr[:, :, 0:512])
            cs_sb = consts.tile([128, S], bf16, tag="cs")
            nc.sync.dma_start(cs_sb, csd[:, :])
            wqk_sb = consts.tile([128, KC, 256], bf16, tag="wqk")
            nc.sync.dma_start(wqk_sb, wqkr)
            cs2_sb = consts.tile([128, S], bf16, tag="cs2")
            nc.sync.dma_start(cs2_sb, cs2d[:, :])
            dj_sb = consts.tile([128, 128], bf16, tag="dj")
            nc.sync.dma_start(dj_sb, djd[:, :])
            pq_sb = consts.tile([128, 128], bf16, tag="pq")
            nc.sync.dma_start(pq_sb, pqd[:, :])
            nc.sync.dma_start(xt_sb[:, :, 512:1024], xtr[:, :, 512:1024])
            wv_sb = consts.tile([128, KC, HD], bf16, tag="wv")
            nc.sync.dma_start(wv_sb, wvd.rearrange("(kc p) m -> p kc m",
                                                   p=128))
            tri_sb = consts.tile([128, 128], bf16, tag="tri")
            nc.sync.dma_start(tri_sb, trid[:, :])
            for n in (2, 3):
                nc.sync.dma_start(xt_sb[:, :, n * 512:(n + 1) * 512],
                                  xtr[:, :, n * 512:(n + 1) * 512])
            id_sb = consts.tile([128, 128], bf16, tag="id")
            nc.sync.dma_start(id_sb, idd[:, :])
            wo_sb = consts.tile([128, 2, H], bf16, tag="wo")
            nc.sync.dma_start(wo_sb, wod.rearrange("(kc p) m -> p kc m",
                                                   p=128))
            oTr = oTd.rearrange("(kc p) m -> p kc m", p=128)

            qhat = [acts.tile([128, S], bf16, tag=f"qh{m}", name=f"qhat{m}")
                    for m in range(G)]
            # raw projections (per head pair) and their rotate-half images
            qtmp = [acts.tile([128, S], bf16, tag=f"qt{p}", name=f"qtmp{p}")
                    for p in range(2)]
            q2sb = [acts.tile([128, S], bf16, tag=f"q2{p}", name=f"q2sb{p}")
                    for p in range(2)]
            khat = acts.tile([128, S], bf16, tag="khat")
            ktmp = acts.tile([128, S], bf16, tag="ktmp")
            v_sb = acts.tile([128, NST, HD + 1], bf16, tag="vsb")
            # normalized attn, stored split by contraction half c so each
            # o-proj chunk needs one contiguous [128,512] DMA-transpose:
            # attn_c[c][:, it*128 + (h%2)*64 :] holds head h = 2c + (h%2)
            attn_c = [anp.tile([128, S], bf16, tag=f"ac{c}", name=f"attnc{c}")
                      for c in range(2)]
            expT = [ep.tile([128, S], bf16, tag=f"e{jt}", name=f"expT{jt}")
                    for jt in range(NST)]
            # second buffer for key-tile 0, alternated by head parity: the
            # next head's first (largest) exp must not wait for the drain's
            # attnV reads of the previous head's expT[0]
            expT0b = ep.tile([128, S], bf16, tag="e0b", name="expT0b")

            def eT(h, jt):
                return expT0b if (jt == 0 and h % 2 == 1) else expT[jt]
            aT = [acts.tile([128, S], bf16, tag=f"aT{c}", name=f"aTc{c}")
                  for c in range(2)]
            etri = [None] * NST

            def k_chunk(n):
                col = n * 512
                ps = pvp.tile([128, 512], f32, tag="v", name="psw")
                for kc in range(KC):
                    nc.tensor.matmul(
                        ps, wk_sb[:, kc, :],
                        xt_sb[:, kc, col:col + 512],
                        start=(kc == 0), stop=(kc == KC - 1))
                nc.vector.tensor_mul(
                    ktmp[:, col:col + 512], ps, cs_sb[:, col:col + 512])

            pp_tiles = {}

            def proj_pair_a(p, n):
                """First half of the pair projection (kc 0..3)."""
                col = n * 512
                ps = pvp.tile([128, 512], f32, tag="v", name="psq")
                pp_tiles[(p, n)] = ps
                for kc in range(KC // 2):
                    nc.tensor.matmul(
                        ps, wqk_sb[:, kc, p * 128:(p + 1) * 128],
                        xt_sb[:, kc, col:col + 512],
                        start=(kc == 0), stop=False)

            def proj_pair_b(p, n):
                """Second half (kc 4..7) + drain to qtmp[p]."""
                col = n * 512
                ps = pp_tiles.pop((p, n))
                for kc in range(KC // 2, KC):
                    nc.tensor.matmul(
                        ps, wqk_sb[:, kc, p * 128:(p + 1) * 128],
                        xt_sb[:, kc, col:col + 512],
                        start=False, stop=(kc == KC - 1))
                nc.vector.tensor_copy(qtmp[p][:, col:col + 512], ps)

            def proj_pair(p, n):
                proj_pair_a(p, n)
                proj_pair_b(p, n)

            def perm_pair(p, n):
                """q2sb[p] = rotate-half(qtmp[p]) with head halves swapped:
                rows 0:64 = rot(q_{2p+1}), rows 64:128 = rot(q_{2p})."""
                col = n * 512
                ps = pvp.tile([128, 512], f32, tag="v", name="psp")
                nc.tensor.matmul(ps, pq_sb, qtmp[p][:, col:col + 512],
                                 start=True, stop=True)
                nc.vector.tensor_copy(q2sb[p][:, col:col + 512], ps)

            def muls_head(h, n):
                """Assemble qhat[h] chunk n from qtmp/q2sb (2 DVE muls).
                h even: qhat = [q*cos ; rot(q)*sin] (normal layout).
                h odd:  qhat = [rot(q)*sin ; q*cos] (half-swapped; scores
                are invariant because khat = [k_rot; k_rot])."""
                p = h // 2
                c0, c1 = n * 512, (n + 1) * 512
                if h % 2 == 0:
                    nc.vector.tensor_mul(
                        qhat[h][0:64, c0:c1], qtmp[p][0:64, c0:c1],
                        cs_sb[0:64, c0:c1])
                    nc.vector.tensor_mul(
                        qhat[h][64:128, c0:c1], q2sb[p][64:128, c0:c1],
                        cs_sb[64:128, c0:c1])
                else:
                    nc.vector.tensor_mul(
                        qhat[h][0:64, c0:c1], q2sb[p][0:64, c0:c1],
                        cs2_sb[0:64, c0:c1])
                    nc.vector.tensor_mul(
                        qhat[h][64:128, c0:c1], qtmp[p][64:128, c0:c1],
                        cs2_sb[64:128, c0:c1])

            def fold_chunk(n):
                col = n * 512
                psf = pvp.tile([128, 512], f32, tag="v", name="psf")
                nc.tensor.matmul(psf, dj_sb, ktmp[:, col:col + 512],
                                 start=True, stop=True)
                nc.vector.tensor_copy(khat[:, col:col + 512], psf)

            def v_proj(st):
                psv = pvp.tile([128, 512], f32, tag="v", name="psv")
                for kc in range(KC):
                    nc.tensor.matmul(
                        psv[:, 0:HD], xt_sb[:, kc, st * 128:(st + 1) * 128],
                        wv_sb[:, kc, :],
                        start=(kc == 0), stop=(kc == KC - 1))
                nc.vector.tensor_copy(v_sb[:, st, 0:HD], psv[:, 0:HD])

            def s_tile(h, jt, t0):
                """Scores + exp for one psum tile (<=512 cols) of key-tile
                jt; t0=0 also makes the diag mask."""
                lo = jt * 128
                tl = min(1024, S - lo - t0)
                lhsT = khat[:, lo:lo + 128]
                sc = scp.tile([128, 1024], f32, tag="sc", name="sc")
                for off in range(0, tl, 512):
                    w = min(512, tl - off)
                    nc.tensor.matmul(
                        sc[:, off:off + w], lhsT,
                        qhat[h][:, lo + t0 + off:lo + t0 + off + w],
                        start=True, stop=True, skip_group_check=True)
                nc.scalar.activation(
                    eT(h, jt)[:, lo + t0:lo + t0 + tl], sc[:, 0:tl],
                    Exp, scale=scale)
                if t0 == 0:
                    et = etp.tile([128, 128], bf16, tag="et", name="et")
                    nc.vector.tensor_mul(et, eT(h, jt)[:, lo:lo + 128],
                                         tri_sb)
                    etri[jt] = et

            def s_piece(h, jt, c0, c1):
                """Scores+exp for absolute i-cols [c0,c1) (<=512 wide) of
                key-tile jt; used to stream head 0 inside the prewave."""
                lo = jt * 128
                lhsT = khat[:, lo:lo + 128]
                sc = scp.tile([128, 1024], f32, tag="sc", name="sc")
                nc.tensor.matmul(sc[:, 0:c1 - c0], lhsT,
                                 qhat[h][:, c0:c1], start=True, stop=True)
                nc.scalar.activation(expT[jt][:, c0:c1], sc[:, 0:c1 - c0],
                                     Exp, scale=scale)
                if c0 == lo:
                    et = etp.tile([128, 128], bf16, tag="et", name="et")
                    nc.vector.tensor_mul(et, expT[jt][:, lo:lo + 128],
                                         tri_sb)
                    etri[jt] = et

            def a_stage(h, it):
                """attnV for query-tile it of head h: accumulate over all
                key tiles jt<=it, then normalize.  Runs during head h+1's
                scores phase; expT[jt<=it] still holds head h's values."""
                sl_lo = it * 128
                av = pav.tile([128, HD + 1], f32, tag="av", name="av")
                for jt in range(it + 1):
                    lhs = (etri[it] if jt == it
                           else eT(h, jt)[:, sl_lo:sl_lo + 128])
                    nc.tensor.matmul(av, lhs, v_sb[:, jt, :],
                                     start=(jt == 0), stop=(jt == it))
                rc = rcp.tile([128, 1], f32, tag="rc", name="rc")
                nc.vector.reciprocal(rc, av[:, HD:HD + 1])
                col = it * 128 + (h % 2) * HD
                nc.vector.tensor_scalar_mul(
                    attn_c[h // 2][:, col:col + HD], av[:, 0:HD], rc)

            def oproj_a_tile(it):
                for c in range(2):
                    psx = pvp.tile([128, 128], bf16, tag="v", name="psx")
                    nc.tensor.transpose(
                        psx, attn_c[c][:, it * 128:(it + 1) * 128], id_sb)
                    nc.vector.tensor_copy(
                        aT[c][:, it * 128:(it + 1) * 128], psx)

            def oproj_a(n):
                for q in range(4):
                    oproj_a_tile(4 * n + q)

            def oproj_b(n, act_help=False, tail=False):
                col = n * 512
                otg = otp.tile([128, KC, 512], bf16, tag="og", name="otg")
                for hc in range(KC):
                    ps2 = pvp.tile([128, 512], f32, tag="v", name="ps2")
                    for kc2 in range(2):
                        nc.tensor.matmul(
                            ps2, wo_sb[:, kc2, hc * 128:(hc + 1) * 128],
                            aT[kc2][:, col:col + 512],
                            start=(kc2 == 0), stop=(kc2 == 1))
                    # ACT helps only where its exp stream has slack
                    if hc % 2 == 1 and act_help:
                        nc.scalar.copy(otg[:, hc, :], ps2)
                    else:
                        nc.vector.tensor_copy(otg[:, hc, :], ps2)
                    nc.sync.dma_start(
                        oTd[hc * 128:(hc + 1) * 128, col:col + 512],
                        otg[:, hc, :])

            # ---- prewave: projections AND all of head 0, streamed
            # chunk-by-chunk as xt arrives.  Piece (jt, n) of head-0 scores
            # covers i-cols [max(jt*128, 512n), 512(n+1)) and is ready as
            # soon as chunk n is projected; attnV a(0,jt) needs only pieces
            # from chunks <= jt//4, so it lags one chunk.
            nc.vector.memset(v_sb[:, :, HD:HD + 1], 1.0)
            for n in range(NSB):
                k_chunk(n)
                fold_chunk(n)
                proj_pair(0, n)
                perm_pair(0, n)
                muls_head(0, n)
                muls_head(1, n)
                for st in range(4 * n, 4 * n + 4):
                    v_proj(st)
                for jt in range(4 * n + 4):
                    c0 = max(jt * 128, 512 * n)
                    s_piece(0, jt, c0, 512 * (n + 1))
                    # attnV of the previous chunk's tiles, interleaved
                    if n > 0 and 4 * (n - 1) <= jt <= 4 * n - 1:
                        a_stage(0, jt)
            for jt in range(4 * (NSB - 1), NST - 2):
                a_stage(0, jt)

            atoms = {1: [], 2: []}
            for n in range(NSB):
                atoms[1].append(lambda n=n: proj_pair_a(1, n))
                atoms[1].append(lambda n=n: proj_pair_b(1, n))
                atoms[1].append(lambda n=n: perm_pair(1, n))
                atoms[1].append(lambda n=n: muls_head(2, n))
            for n in range(NSB):
                atoms[2].append(lambda n=n: muls_head(3, n))

            # ---- heads 1-3: 2-stage software pipeline ----
            # head 0's last two attnV stages carry into head 1, like the
            # other head boundaries
            from collections import deque
            pending = deque([(0, NST - 2), (0, NST - 1)])
            for h in range(1, G):
                for jt in range(NST):
                    if atoms.get(h) and jt >= 1:
                        atoms[h].pop(0)()
                        # late stages are cheap on both PE and ACT: take two
                        if jt >= 13 and atoms[h]:
                            atoms[h].pop(0)()
                    s_tile(h, jt, 0)
                    # pop a previous-head leftover immediately (it must run
                    # before this head's stage-1 overwrites expT[1])
                    if pending and (pending[0][0] < h
                                    or len(pending) >=
                                    (2 if h == G - 1 else 3)):
                        a_stage(*pending.popleft())
                    if S - jt * 128 > 1024:
                        s_tile(h, jt, 1024)
                    pending.append((h, jt))
                    if h == G - 1 and jt in (5, 9, 13):
                        oproj_a((jt - 5) // 4)
                    if h == G - 1 and jt in (6, 10, 14):
                        oproj_b((jt - 6) // 4, act_help=(jt >= 10))
                    # group 3's first transposes, as soon as their attnV
                    # stages have popped
                    if h == G - 1 and jt in (14, 15):
                        oproj_a_tile(jt - 2)
                # carry one attnV stage into the next head: it only reads
                # this head's expT, and the next head's first write goes to
                # the other jt=0 buffer
                while len(pending) > 2:
                    a_stage(*pending.popleft())
            # tail: interleave the final two attnV stages with the last
            # group's transposes instead of serializing them
            a_stage(*pending.popleft())
            oproj_a_tile(NST - 2)
            a_stage(*pending.popleft())
            oproj_a_tile(NST - 1)
            oproj_b(NSB - 1, act_help=True, tail=True)

            if debug:
                for c in range(2):
                    nc.sync.dma_start(dbg_attn[c], attn_c[c])
                    nc.sync.dma_start(dbg_aT[c], aT[c])
                for jt in range(NST):
                    nc.sync.dma_start(dbg_exp[jt], expT[jt])
                for m in range(G):
                    nc.sync.dma_start(dbg_qk[m], qhat[m])
                nc.sync.dma_start(dbg_qk[G], khat)
                nc.sync.dma_start(dbg_v[:, :, :], v_sb)

    nc.finalize()
    return nc


def _host_inputs(hidden_states, position_ids, wq, wk, wv, wo):
    """Build the 8 per-core input maps."""
    def w2_of(w):
        # w: [64, H] rows of one head; returns sign-permuted rows
        w2 = np.empty_like(w)
        w2[:32] = -w[32:64]
        w2[32:] = w[:32]
        return w2

    dupJ = np.zeros((128, 128), np.float32)
    for p in range(128):
        dupJ[p, p % 64] = 1.0
        dupJ[p, p % 64 + 64] = 1.0
    dupJ = dupJ.astype(BF16)
    ident = np.eye(128, dtype=np.float32).astype(BF16)
    trimask = np.triu(np.ones((128, 128), np.float32)).astype(BF16)

    # permQ: q2sb = permQ.T @ [q_even; q_odd] gives
    # rows 0:64 = rot(q_odd), rows 64:128 = rot(q_even)
    permQ = np.zeros((128, 128), np.float32)
    for m in range(0, 32):
        permQ[96 + m, m] = -1.0
    for m in range(32, 64):
        permQ[m + 32, m] = 1.0
    for m in range(64, 96):
        permQ[m - 32, m] = -1.0
    for m in range(96, 128):
        permQ[m - 96, m] = 1.0
    permQ = permQ.astype(BF16)

    in_maps = []
    for core in range(N_CORES):
        b, kv = core // NKV, core % NKV
        xT = np.ascontiguousarray(hidden_states[b].T).astype(BF16)

        cols = []
        for i in range(G):
            h = kv * G + i
            cols.append(wq[h * HD:(h + 1) * HD].T)
        wqkT = np.ascontiguousarray(np.concatenate(cols, axis=1)).astype(BF16)
        wkh = wk[kv * HD:(kv + 1) * HD]
        wkcols = np.concatenate([wkh.T, w2_of(wkh).T], axis=1)  # [H, 128]
        wkT2 = np.ascontiguousarray(
            wkcols.reshape(8, 128, 128).transpose(1, 0, 2).reshape(128, 1024)
        ).astype(BF16)

        wvT = np.ascontiguousarray(wv[kv * HD:(kv + 1) * HD].T).astype(BF16)
        woT = np.ascontiguousarray(
            wo[:, kv * G * HD:(kv + 1) * G * HD].T).astype(BF16)

        inv = 1.0 / (THETA ** (np.arange(0, HD, 2, dtype=np.float32) / HD))
        freqs = position_ids[b].astype(np.float32)[:, None] * inv[None, :]
        emb = np.concatenate([freqs, freqs], axis=-1)       # [S, 64]
        cosT, sinT = np.cos(emb).T, np.sin(emb).T
        cs = np.ascontiguousarray(
            np.concatenate([cosT, sinT], axis=0)).astype(BF16)
        cs2 = np.ascontiguousarray(
            np.concatenate([sinT, cosT], axis=0)).astype(BF16)

        in_maps.append({
            "xT": xT, "wqkT": wqkT, "wkT2": wkT2, "wvT": wvT,
            "cs": cs, "cs2": cs2,
            "woT": woT, "dupJ": dupJ, "permQ": permQ, "ident": ident,
            "trimask": trimask,
        })
    return in_maps


_NC_CACHE = {}


def run_cores(in_maps, trace=False, trace_kwargs=None, debug=False):
    from concourse.bass_utils import run_bass_kernel_spmd
    key = "nc_dbg" if debug else "nc"
    if key not in _NC_CACHE:
        _NC_CACHE[key] = _build_nc(debug=debug)
    nc = _NC_CACHE[key]
    return run_bass_kernel_spmd(
        nc, in_maps, core_ids=list(range(N_CORES)),
        trace=trace, **(trace_kwargs or {}))


def kernel(hidden_states, attention_mask, position_ids, wq, wk, wv, wo):
    hidden_states = np.asarray(hidden_states, dtype=np.float32)
    position_ids = np.asarray(position_ids)
    wq = np.asarray(wq, dtype=np.float32)
    wk = np.asarray(wk, dtype=np.float32)
    wv = np.asarray(wv, dtype=np.float32)
    wo = np.asarray(wo, dtype=np.float32)

    in_maps = _host_inputs(hidden_states, position_ids, wq, wk, wv, wo)
    res = run_cores(in_maps)

    out = np.zeros((B, S, H), np.float32)
    for core in range(N_CORES):
        b = core // NKV
        out[b] += res.results[core]["oT"].T.astype(np.float32)
    return out


# revision 80
# speedup vs baseline: 1.0025x; 1.0025x over previous
"""Trainium2 Bass kernel for MimiAttention (GQA + RoPE + causal softmax).

Problem: B=2, S=2048, H=1024, NH=16 q-heads, NKV=4 kv-heads, HD=64.
Sharding: 8 cores = 2 (batch) x 4 (kv-group).  Each core computes one batch's
attention for one GQA group (4 q-heads sharing 1 kv head) and the partial
o-projection for those heads; the host sums the 4 partials per batch.

Per-core device pipeline (all matmuls bf16 in / fp32 psum out):
  1. QKV projection in [d, s] layout.  RoPE is realized without any
     cross-partition shuffles by computing a second projection with
     sign-permuted weight rows (W2 rows: d<32 -> -W[d+32], d>=32 -> W[d-32]):
       q_rot = q*cos + q2*sin
     The scores contraction then uses the 128-dim identity
       q_rot . k_rot = concat(q*cos, q2*sin) . concat(k_rot, k_rot)
     so Qhat = [q*cos; q2*sin] needs only ONE elementwise multiply per chunk,
     and Khat = [k_rot; k_rot] is built by one matmul with the fold matrix
     J[p,m] = (p == m mod 64).
  2. Scores computed TRANSPOSED (scoresT[j,i]) per key-tile jt into wide
     [128,1024] psum tiles; ONE exp per tile on ACT with the 1/sqrt(64)
     scale folded in (no max subtraction needed: |s*scale|<~3); causal
     triangle handled by a bf16 mask multiply on DVE for the diagonal tile.
  3. attnV flipped: for each query tile `it`, accumulate over ALL key tiles
     jt<=it into a small [128,65] psum ring (column 64 = ones -> softmax
     denominator), then reciprocal+scale on DVE.  Short psum residency
     frees banks for double-buffered scores and o-proj.
  4. Software pipelining: attnV for tile it is issued two stages behind the
     scores for tile it+2, so PE never stalls on ACT's exp latency.
  5. o-projection per 512-col chunk: PE-transpose attn [i,c] -> [c,i]
     (4 transposes batched per psum tile), 2-slot psum ring, output copies
     alternate DVE/ACT, one batched DMA per chunk ([128,8,512] -> oT).
"""

import numpy as np
import ml_dtypes

B, S, H = 2, 2048, 1024
NH, NKV, HD = 16, 4, 64
G = NH // NKV            # 4 q-heads per kv head
THETA = 10000.0
N_CORES = 8

BF16 = ml_dtypes.bfloat16


def _build_nc(debug=False):
    import concourse.mybir as mybir
    import concourse.tile as tile
    from concourse import bacc

    f32 = mybir.dt.float32
    bf16 = mybir.dt.bfloat16

    nc = bacc.Bacc("TRN2", target_bir_lowering=False)

    xTd = nc.dram_tensor("xT", [H, S], bf16, kind="ExternalInput")
    wqkd = nc.dram_tensor("wqkT", [H, 256], bf16, kind="ExternalInput")
    wkd = nc.dram_tensor("wkT2", [128, 1024], bf16, kind="ExternalInput")
    wvd = nc.dram_tensor("wvT", [H, HD], bf16, kind="ExternalInput")
    csd = nc.dram_tensor("cs", [128, S], bf16, kind="ExternalInput")
    cs2d = nc.dram_tensor("cs2", [128, S], bf16, kind="ExternalInput")
    wod = nc.dram_tensor("woT", [G * HD, H], bf16, kind="ExternalInput")
    djd = nc.dram_tensor("dupJ", [128, 128], bf16, kind="ExternalInput")
    pqd = nc.dram_tensor("permQ", [128, 128], bf16, kind="ExternalInput")
    idd = nc.dram_tensor("ident", [128, 128], bf16, kind="ExternalInput")
    trid = nc.dram_tensor("trimask", [128, 128], bf16, kind="ExternalInput")
    oTd = nc.dram_tensor("oT", [H, S], bf16, kind="ExternalOutput")
    if debug:
        dbg_attn = nc.dram_tensor("dbg_attn", [2, 128, S], bf16,
                                  kind="ExternalOutput")
        dbg_exp = nc.dram_tensor("dbg_exp", [16, 128, S], bf16,
                                 kind="ExternalOutput")
        dbg_qk = nc.dram_tensor("dbg_qk", [G + 1, 128, S], bf16,
                                kind="ExternalOutput")
        dbg_v = nc.dram_tensor("dbg_v", [128, 16, HD + 1], bf16,
                               kind="ExternalOutput")
        dbg_aT = nc.dram_tensor("dbg_aT", [2, 128, S], bf16,
                                kind="ExternalOutput")

    NSB = S // 512        # 4 chunks of 512
    NST = S // 128        # 16 tiles of 128
    KC = H // 128         # 8 contraction chunks
    scale = float(1.0 / np.sqrt(HD))
    Exp = mybir.ActivationFunctionType.Exp

    with tile.TileContext(nc) as tc:
        import contextlib
        ctx = contextlib.ExitStack()
        with ctx:
            consts = ctx.enter_context(tc.tile_pool(name="consts", bufs=1))
            acts = ctx.enter_context(tc.tile_pool(name="acts", bufs=1))
            anp = ctx.enter_context(tc.tile_pool(name="attn", bufs=1))
            rcp = ctx.enter_context(tc.tile_pool(name="rcp", bufs=6))
            etp = ctx.enter_context(tc.tile_pool(name="etri", bufs=8))
            ep = ctx.enter_context(tc.tile_pool(name="exps", bufs=1))
            otp = ctx.enter_context(tc.tile_pool(name="ot", bufs=2))
            # PSUM: scores 2x[128,1024]f32 (4 banks) + attnV accum
            # 2x[128,65] (2 banks) + shared proj/oproj ring 2x[128,512]
            # (2 banks) = 8 banks.
            scp = ctx.enter_context(
                tc.tile_pool(name="sc", bufs=2, space="PSUM"))
            pav = ctx.enter_context(
                tc.tile_pool(name="av", bufs=2, space="PSUM"))
            pvp = ctx.enter_context(
                tc.tile_pool(name="vr", bufs=2, space="PSUM"))

            # ---- input DMAs, ordered by first use; xt arrives in 512-col
            # chunks (all KC row-blocks per chunk) so the prewave can start
            # after ~2 transfers.
            xtr = xTd.rearrange("(kc p) m -> p kc m", p=128)
            wqkr = wqkd.rearrange("(kc p) m -> p kc m", p=128)
            wk_sb = consts.tile([128, KC, 128], bf16, tag="wk")
            nc.sync.dma_start(wk_sb, wkd.rearrange("p (kc m) -> p kc m",
                                                   kc=KC))
            xt_sb = consts.tile([128, KC, S], bf16, tag="xt")
            nc.sync.dma_start(xt_sb[:, :, 0:512], xtr[:, :, 0:512])
            cs_sb = consts.tile([128, S], bf16, tag="cs")
            nc.sync.dma_start(cs_sb, csd[:, :])
            wqk_sb = consts.tile([128, KC, 256], bf16, tag="wqk")
            nc.sync.dma_start(wqk_sb, wqkr)
            cs2_sb = consts.tile([128, S], bf16, tag="cs2")
            nc.sync.dma_start(cs2_sb, cs2d[:, :])
            dj_sb = consts.tile([128, 128], bf16, tag="dj")
            nc.sync.dma_start(dj_sb, djd[:, :])
            pq_sb = consts.tile([128, 128], bf16, tag="pq")
            nc.sync.dma_start(pq_sb, pqd[:, :])
            nc.sync.dma_start(xt_sb[:, :, 512:1024], xtr[:, :, 512:1024])
            wv_sb = consts.tile([128, KC, HD], bf16, tag="wv")
            nc.sync.dma_start(wv_sb, wvd.rearrange("(kc p) m -> p kc m",
                                                   p=128))
            tri_sb = consts.tile([128, 128], bf16, tag="tri")
            nc.sync.dma_start(tri_sb, trid[:, :])
            for n in (2, 3):
                nc.sync.dma_start(xt_sb[:, :, n * 512:(n + 1) * 512],
                                  xtr[:, :, n * 512:(n + 1) * 512])
            id_sb = consts.tile([128, 128], bf16, tag="id")
            nc.sync.dma_start(id_sb, idd[:, :])
            wo_sb = consts.tile([128, 2, H], bf16, tag="wo")
            nc.sync.dma_start(wo_sb, wod.rearrange("(kc p) m -> p kc m",
                                                   p=128))
            oTr = oTd.rearrange("(kc p) m -> p kc m", p=128)

            qhat = [acts.tile([128, S], bf16, tag=f"qh{m}", name=f"qhat{m}")
                    for m in range(G)]
            # raw projections (per head pair) and their rotate-half images
            qtmp = [acts.tile([128, S], bf16, tag=f"qt{p}", name=f"qtmp{p}")
                    for p in range(2)]
            q2sb = [acts.tile([128, S], bf16, tag=f"q2{p}", name=f"q2sb{p}")
                    for p in range(2)]
            khat = acts.tile([128, S], bf16, tag="khat")
            ktmp = acts.tile([128, S], bf16, tag="ktmp")
            v_sb = acts.tile([128, NST, HD + 1], bf16, tag="vsb")
            # normalized attn, stored split by contraction half c so each
            # o-proj chunk needs one contiguous [128,512] DMA-transpose:
            # attn_c[c][:, it*128 + (h%2)*64 :] holds head h = 2c + (h%2)
            attn_c = [anp.tile([128, S], bf16, tag=f"ac{c}", name=f"attnc{c}")
                      for c in range(2)]
            expT = [ep.tile([128, S], bf16, tag=f"e{jt}", name=f"expT{jt}")
                    for jt in range(NST)]
            # second buffer for key-tile 0, alternated by head parity: the
            # next head's first (largest) exp must not wait for the drain's
            # attnV reads of the previous head's expT[0]
            expT0b = ep.tile([128, S], bf16, tag="e0b", name="expT0b")

            def eT(h, jt):
                return expT0b if (jt == 0 and h % 2 == 1) else expT[jt]
            aT = [acts.tile([128, S], bf16, tag=f"aT{c}", name=f"aTc{c}")
                  for c in range(2)]
            etri = [None] * NST

            def k_chunk(n):
                col = n * 512
                ps = pvp.tile([128, 512], f32, tag="v", name="psw")
                for kc in range(KC):
                    nc.tensor.matmul(
                        ps, wk_sb[:, kc, :],
                        xt_sb[:, kc, col:col + 512],
                        start=(kc == 0), stop=(kc == KC - 1))
                nc.vector.tensor_mul(
                    ktmp[:, col:col + 512], ps, cs_sb[:, col:col + 512])

            pp_tiles = {}

            def proj_pair_a(p, n):
                """First half of the pair projection (kc 0..3)."""
                col = n * 512
                ps = pvp.tile([128, 512], f32, tag="v", name="psq")
                pp_tiles[(p, n)] = ps
                for kc in range(KC // 2):
                    nc.tensor.matmul(
                        ps, wqk_sb[:, kc, p * 128:(p + 1) * 128],
                        xt_sb[:, kc, col:col + 512],
                        start=(kc == 0), stop=False)

            def proj_pair_b(p, n):
                """Second half (kc 4..7) + drain to qtmp[p]."""
                col = n * 512
                ps = pp_tiles.pop((p, n))
                for kc in range(KC // 2, KC):
                    nc.tensor.matmul(
                        ps, wqk_sb[:, kc, p * 128:(p + 1) * 128],
                        xt_sb[:, kc, col:col + 512],
                        start=False, stop=(kc == KC - 1))
                nc.vector.tensor_copy(qtmp[p][:, col:col + 512], ps)

            def proj_pair(p, n):
                proj_pair_a(p, n)
                proj_pair_b(p, n)

            def perm_pair(p, n):
                """q2sb[p] = rotate-half(qtmp[p]) with head halves swapped:
                rows 0:64 = rot(q_{2p+1}), rows 64:128 = rot(q_{2p})."""
                col = n * 512
                ps = pvp.tile([128, 512], f32, tag="v", name="psp")
                nc.tensor.matmul(ps, pq_sb, qtmp[p][:, col:col + 512],
                                 start=True, stop=True)
                nc.vector.tensor_copy(q2sb[p][:, col:col + 512], ps)

            def muls_head(h, n):
                """Assemble qhat[h] chunk n from qtmp/q2sb (2 DVE muls).
                h even: qhat = [q*cos ; rot(q)*sin] (normal layout).
                h odd:  qhat = [rot(q)*sin ; q*cos] (half-swapped; scores
                are invariant because khat = [k_rot; k_rot])."""
                p = h // 2
                c0, c1 = n * 512, (n + 1) * 512
                if h % 2 == 0:
                    nc.vector.tensor_mul(
                        qhat[h][0:64, c0:c1], qtmp[p][0:64, c0:c1],
                        cs_sb[0:64, c0:c1])
                    nc.vector.tensor_mul(
                        qhat[h][64:128, c0:c1], q2sb[p][64:128, c0:c1],
                        cs_sb[64:128, c0:c1])
                else:
                    nc.vector.tensor_mul(
                        qhat[h][0:64, c0:c1], q2sb[p][0:64, c0:c1],
                        cs2_sb[0:64, c0:c1])
                    nc.vector.tensor_mul(
                        qhat[h][64:128, c0:c1], qtmp[p][64:128, c0:c1],
                        cs2_sb[64:128, c0:c1])

            def fold_chunk(n):
                col = n * 512
                psf = pvp.tile([128, 512], f32, tag="v", name="psf")
                nc.tensor.matmul(psf, dj_sb, ktmp[:, col:col + 512],
                                 start=True, stop=True)
                nc.vector.tensor_copy(khat[:, col:col + 512], psf)

            def v_proj(st):
                psv = pvp.tile([128, 512], f32, tag="v", name="psv")
                for kc in range(KC):
                    nc.tensor.matmul(
                        psv[:, 0:HD], xt_sb[:, kc, st * 128:(st + 1) * 128],
                        wv_sb[:, kc, :],
                        start=(kc == 0), stop=(kc == KC - 1))
                nc.vector.tensor_copy(v_sb[:, st, 0:HD], psv[:, 0:HD])

            def s_tile(h, jt, t0):
                """Scores + exp for one psum tile (<=512 cols) of key-tile
                jt; t0=0 also makes the diag mask."""
                lo = jt * 128
                tl = min(1024, S - lo - t0)
                lhsT = khat[:, lo:lo + 128]
                sc = scp.tile([128, 1024], f32, tag="sc", name="sc")
                for off in range(0, tl, 512):
                    w = min(512, tl - off)
                    nc.tensor.matmul(
                        sc[:, off:off + w], lhsT,
                        qhat[h][:, lo + t0 + off:lo + t0 + off + w],
                        start=True, stop=True, skip_group_check=True)
                nc.scalar.activation(
                    eT(h, jt)[:, lo + t0:lo + t0 + tl], sc[:, 0:tl],
                    Exp, scale=scale)
                if t0 == 0:
                    et = etp.tile([128, 128], bf16, tag="et", name="et")
                    nc.vector.tensor_mul(et, eT(h, jt)[:, lo:lo + 128],
                                         tri_sb)
                    etri[jt] = et

            def s_piece(h, jt, c0, c1):
                """Scores+exp for absolute i-cols [c0,c1) (<=512 wide) of
                key-tile jt; used to stream head 0 inside the prewave."""
                lo = jt * 128
                lhsT = khat[:, lo:lo + 128]
                sc = scp.tile([128, 1024], f32, tag="sc", name="sc")
                nc.tensor.matmul(sc[:, 0:c1 - c0], lhsT,
                                 qhat[h][:, c0:c1], start=True, stop=True)
                nc.scalar.activation(expT[jt][:, c0:c1], sc[:, 0:c1 - c0],
                                     Exp, scale=scale)
                if c0 == lo:
                    et = etp.tile([128, 128], bf16, tag="et", name="et")
                    nc.vector.tensor_mul(et, expT[jt][:, lo:lo + 128],
                                         tri_sb)
                    etri[jt] = et

            def a_stage(h, it):
                """attnV for query-tile it of head h: accumulate over all
                key tiles jt<=it, then normalize.  Runs during head h+1's
                scores phase; expT[jt<=it] still holds head h's values."""
                sl_lo = it * 128
                av = pav.tile([128, HD + 1], f32, tag="av", name="av")
                for jt in range(it + 1):
                    lhs = (etri[it] if jt == it
                           else eT(h, jt)[:, sl_lo:sl_lo + 128])
                    nc.tensor.matmul(av, lhs, v_sb[:, jt, :],
                                     start=(jt == 0), stop=(jt == it))
                rc = rcp.tile([128, 1], f32, tag="rc", name="rc")
                nc.vector.reciprocal(rc, av[:, HD:HD + 1])
                col = it * 128 + (h % 2) * HD
                nc.vector.tensor_scalar_mul(
                    attn_c[h // 2][:, col:col + HD], av[:, 0:HD], rc)

            def oproj_a_tile(it):
                for c in range(2):
                    psx = pvp.tile([128, 128], bf16, tag="v", name="psx")
                    nc.tensor.transpose(
                        psx, attn_c[c][:, it * 128:(it + 1) * 128], id_sb)
                    nc.vector.tensor_copy(
                        aT[c][:, it * 128:(it + 1) * 128], psx)

            def oproj_a(n):
                for q in range(4):
                    oproj_a_tile(4 * n + q)

            def oproj_b(n, act_help=False, tail=False):
                col = n * 512
                otg = otp.tile([128, KC, 512], bf16, tag="og", name="otg")
                for hc in range(KC):
                    ps2 = pvp.tile([128, 512], f32, tag="v", name="ps2")
                    for kc2 in range(2):
                        nc.tensor.matmul(
                            ps2, wo_sb[:, kc2, hc * 128:(hc + 1) * 128],
                            aT[kc2][:, col:col + 512],
                            start=(kc2 == 0), stop=(kc2 == 1))
                    # ACT helps only where its exp stream has slack
                    if hc % 2 == 1 and act_help:
                        nc.scalar.copy(otg[:, hc, :], ps2)
                    else:
                        nc.vector.tensor_copy(otg[:, hc, :], ps2)
                    nc.sync.dma_start(
                        oTd[hc * 128:(hc + 1) * 128, col:col + 512],
                        otg[:, hc, :])

            # ---- prewave: projections AND all of head 0, streamed
            # chunk-by-chunk as xt arrives.  Piece (jt, n) of head-0 scores
            # covers i-cols [max(jt*128, 512n), 512(n+1)) and is ready as
            # soon as chunk n is projected; attnV a(0,jt) needs only pieces
            # from chunks <= jt//4, so it lags one chunk.
            nc.vector.memset(v_sb[:, :, HD:HD + 1], 1.0)
            for n in range(NSB):
                k_chunk(n)
                fold_chunk(n)
                proj_pair(0, n)
                perm_pair(0, n)
                muls_head(0, n)
                muls_head(1, n)
                for st in range(4 * n, 4 * n + 4):
                    v_proj(st)
                for jt in range(4 * n + 4):
                    c0 = max(jt * 128, 512 * n)
                    s_piece(0, jt, c0, 512 * (n + 1))
                    # attnV of the previous chunk's tiles, interleaved
                    if n > 0 and 4 * (n - 1) <= jt <= 4 * n - 1:
                        a_stage(0, jt)
            for jt in range(4 * (NSB - 1), NST - 2):
                a_stage(0, jt)

            atoms = {1: [], 2: []}
            for n in range(NSB):
                atoms[1].append(lambda n=n: proj_pair_a(1, n))
                atoms[1].append(lambda n=n: proj_pair_b(1, n))
                atoms[1].append(lambda n=n: perm_pair(1, n))
                atoms[1].append(lambda n=n: muls_head(2, n))
            for n in range(NSB):
                atoms[2].append(lambda n=n: muls_head(3, n))

            # ---- heads 1-3: 2-stage software pipeline ----
            # head 0's last two attnV stages carry into head 1, like the
            # other head boundaries
            from collections import deque
            pending = deque([(0, NST - 2), (0, NST - 1)])
            for h in range(1, G):
                for jt in range(NST):
                    if atoms.get(h) and jt >= 1:
                        atoms[h].pop(0)()
                        # late stages are cheap on both PE and ACT: take two
                        if jt >= 13 and atoms[h]:
                            atoms[h].pop(0)()
                    two = S - jt * 128 > 1024
                    if two:
                        s_tile(h, jt, 0)
                    if pending and (pending[0][0] < h
                                    or len(pending) >=
                                    (2 if h == G - 1 else 3)):
                        a_stage(*pending.popleft())
                    if two:
                        s_tile(h, jt, 1024)
                    else:
                        s_tile(h, jt, 0)
                    pending.append((h, jt))
                    if h == G - 1 and jt in (5, 9, 13):
                        oproj_a((jt - 5) // 4)
                    if h == G - 1 and jt in (6, 10, 14):
                        oproj_b((jt - 6) // 4, act_help=(jt >= 10))
                    # group 3's first transposes, as soon as their attnV
                    # stages have popped
                    if h == G - 1 and jt in (14, 15):
                        oproj_a_tile(jt - 2)
                # carry one attnV stage into the next head: it only reads
                # this head's expT, and the next head's first write goes to
                # the other jt=0 buffer
                while len(pending) > 2:
                    a_stage(*pending.popleft())
            # tail: interleave the final two attnV stages with the last
            # group's transposes instead of serializing them
            a_stage(*pending.popleft())
            oproj_a_tile(NST - 2)
            a_stage(*pending.popleft())
            oproj_a_tile(NST - 1)
            oproj_b(NSB - 1, act_help=True, tail=True)

            if debug:
                for c in range(2):
                    nc.sync.dma_start(dbg_attn[c], attn_c[c])
                    nc.sync.dma_start(dbg_aT[c], aT[c])
                for jt in range(NST):
                    nc.sync.dma_start(dbg_exp[jt], expT[jt])
                for m in range(G):
                    nc.sync.dma_start(dbg_qk[m], qhat[m])
                nc.sync.dma_start(dbg_qk[G], khat)
                nc.sync.dma_start(dbg_v[:, :, :], v_sb)

    nc.finalize()
    return nc


def _host_inputs(hidden_states, position_ids, wq, wk, wv, wo):
    """Build the 8 per-core input maps."""
    def w2_of(w):
        # w: [64, H] rows of one head; returns sign-permuted rows
        w2 = np.empty_like(w)
        w2[:32] = -w[32:64]
        w2[32:] = w[:32]
        return w2

    dupJ = np.zeros((128, 128), np.float32)
    for p in range(128):
        dupJ[p, p % 64] = 1.0
        dupJ[p, p % 64 + 64] = 1.0
    dupJ = dupJ.astype(BF16)
    ident = np.eye(128, dtype=np.float32).astype(BF16)
    trimask = np.triu(np.ones((128, 128), np.float32)).astype(BF16)

    # permQ: q2sb = permQ.T @ [q_even; q_odd] gives
    # rows 0:64 = rot(q_odd), rows 64:128 = rot(q_even)
    permQ = np.zeros((128, 128), np.float32)
    for m in range(0, 32):
        permQ[96 + m, m] = -1.0
    for m in range(32, 64):
        permQ[m + 32, m] = 1.0
    for m in range(64, 96):
        permQ[m - 32, m] = -1.0
    for m in range(96, 128):
        permQ[m - 96, m] = 1.0
    permQ = permQ.astype(BF16)

    in_maps = []
    for core in range(N_CORES):
        b, kv = core // NKV, core % NKV
        xT = np.ascontiguousarray(hidden_states[b].T).astype(BF16)

        cols = []
        for i in range(G):
            h = kv * G + i
            cols.append(wq[h * HD:(h + 1) * HD].T)
        wqkT = np.ascontiguousarray(np.concatenate(cols, axis=1)).astype(BF16)
        wkh = wk[kv * HD:(kv + 1) * HD]
        wkcols = np.concatenate([wkh.T, w2_of(wkh).T], axis=1)  # [H, 128]
        wkT2 = np.ascontiguousarray(
            wkcols.reshape(8, 128, 128).transpose(1, 0, 2).reshape(128, 1024)
        ).astype(BF16)

        wvT = np.ascontiguousarray(wv[kv * HD:(kv + 1) * HD].T).astype(BF16)
        woT = np.ascontiguousarray(
            wo[:, kv * G * HD:(kv + 1) * G * HD].T).astype(BF16)

        inv = 1.0 / (THETA ** (np.arange(0, HD, 2, dtype=np.float32) / HD))
        freqs = position_ids[b].astype(np.float32)[:, None] * inv[None, :]
        emb = np.concatenate([freqs, freqs], axis=-1)       # [S, 64]
        cosT, sinT = np.cos(emb).T, np.sin(emb).T
        cs = np.ascontiguousarray(
            np.concatenate([cosT, sinT], axis=0)).astype(BF16)
        cs2 = np.ascontiguousarray(
            np.concatenate([sinT, cosT], axis=0)).astype(BF16)

        in_maps.append({
            "xT": xT, "wqkT": wqkT, "wkT2": wkT2, "wvT": wvT,
            "cs": cs, "cs2": cs2,
            "woT": woT, "dupJ": dupJ, "permQ": permQ, "ident": ident,
            "trimask": trimask,
        })
    return in_maps


_NC_CACHE = {}


def run_cores(in_maps, trace=False, trace_kwargs=None, debug=False):
    from concourse.bass_utils import run_bass_kernel_spmd
    key = "nc_dbg" if debug else "nc"
    if key not in _NC_CACHE:
        _NC_CACHE[key] = _build_nc(debug=debug)
    nc = _NC_CACHE[key]
    return run_bass_kernel_spmd(
        nc, in_maps, core_ids=list(range(N_CORES)),
        trace=trace, **(trace_kwargs or {}))


def kernel(hidden_states, attention_mask, position_ids, wq, wk, wv, wo):
    hidden_states = np.asarray(hidden_states, dtype=np.float32)
    position_ids = np.asarray(position_ids)
    wq = np.asarray(wq, dtype=np.float32)
    wk = np.asarray(wk, dtype=np.float32)
    wv = np.asarray(wv, dtype=np.float32)
    wo = np.asarray(wo, dtype=np.float32)

    in_maps = _host_inputs(hidden_states, position_ids, wq, wk, wv, wo)
    res = run_cores(in_maps)

    out = np.zeros((B, S, H), np.float32)
    for core in range(N_CORES):
        b = core // NKV
        out[b] += res.results[core]["oT"].T.astype(np.float32)
    return out
